# revision 17
# baseline (speedup 1.0000x reference)
"""Trainium2 Bass kernel for nn_Attention (dense_transformer).

Math (per fused-batch element, 32 total = b*m):
    qkv = x @ w_qkv ; split q,k,v into 8 heads of 64
    sim = (q/8) @ k^T  (+ pos_bias term that is constant along the softmax
                        axis -> provably no effect on softmax output, dropped)
    attn = softmax(sim); out = (attn @ v) heads-concat @ w_out

Sharding: pure data-parallel over the fused (b*m)=32 axis -> 4 elements
per core on 8 cores, no collectives. Weights replicated.

Kernel strategy (per core, all-transposed dataflow, bf16 matmuls):
    xT   = PE-transpose(x)                        [c, n]
    qT,kT (pair-stacked) = W_qk^T @ xT            [e_slice, n]  (psum f32)
    V    = xT-slices @ W_v                        [n, e_v] natural layout,
           stored interleaved [n, h, 65] with a ones column per head
    S^T  = kT_h^T-slice @ qT_h                    [j, i] per head
    P^T  = exp(0.125 * S^T)  (no max subtraction: |logits| <= ~8)
    outT_h (rows 0..63) + L_h (row 64) = V1_h^T @ P^T   (ones-column trick)
    OT   = outT_h * broadcast(1/L_h)  (K=1 matmul broadcast + DVE mul)
    out  = OT-slices^T @ w_out        [n, c] -> DMA out
"""

import os
import sys

for _p in ("/root/.axon_site/_ro/trn_rl_repo", "/opt/trn_rl_repo"):
    if os.path.isdir(_p) and _p not in sys.path:
        sys.path.append(_p)

import numpy as np

# ---- problem constants (hardcoded per spec) ----
B, M, N, C = 4, 8, 512, 512
HEADS, DHEAD = 8, 64
E3 = 3 * 512
NCORES = 8
BPC = (B * M) // NCORES  # batch elements per core = 4
BCAST_MODE = "gpsimd"  # "gpsimd" | "pe"
RECIP_MODE = "approx_sbuf"  # "exact" | "approx_sbuf" | "approx_psum"
TR_MODE = "pe"  # "dma" (xbar transpose) | "pe" (tensor-engine transpose)
ACT_COPIES = False  # offload some psum->sbuf copies to the Scalar engine

_cache = {}


def _build():
    import concourse.bass as bass
    import concourse.mybir as mybir
    import concourse.tile as tile
    from concourse import bacc
    from concourse.masks import make_identity

    f32 = mybir.dt.float32
    bf16 = mybir.dt.bfloat16
    f32r = mybir.dt.float32r
    EXP = mybir.ActivationFunctionType.Exp

    nc = bacc.Bacc("TRN2", target_bir_lowering=False, debug=False,
                   num_devices=NCORES)

    x_ext = nc.declare_dram_parameter("x", [BPC, N, C], f32, isOutput=False)
    wq_ext = nc.declare_dram_parameter("w_qkv", [C, E3], f32, isOutput=False)
    wo_ext = nc.declare_dram_parameter("w_out", [512, 512], f32, isOutput=False)
    out_ext = nc.declare_dram_parameter("out", [BPC, N, C], f32, isOutput=True)

    from contextlib import ExitStack

    with tile.TileContext(nc) as tc, ExitStack() as ctx:
        # ---------------- pools ----------------
        p_const = ctx.enter_context(tc.tile_pool(name="const", bufs=1))
        p_stage = ctx.enter_context(tc.tile_pool(name="stage", bufs=1))
        p_x = ctx.enter_context(tc.tile_pool(name="x", bufs=2))
        p_xT = ctx.enter_context(tc.tile_pool(name="xT", bufs=2))
        p_qk = ctx.enter_context(tc.tile_pool(name="qk", bufs=2))
        p_v = ctx.enter_context(tc.tile_pool(name="v", bufs=2))
        p_pt = ctx.enter_context(tc.tile_pool(name="pt", bufs=4))
        p_oT = ctx.enter_context(tc.tile_pool(name="oT", bufs=2))
        p_out = ctx.enter_context(tc.tile_pool(name="out", bufs=2))
        p_small = ctx.enter_context(tc.tile_pool(name="small", bufs=4))

        # tr and st share one pool (same tag) so STs can run well ahead of
        # the ACT exps; PSUM slots are allocated dynamically from the shared
        # 8-bank free pool, so nominal bufs sums may exceed 8
        ps_st = ctx.enter_context(tc.tile_pool(name="ps_st", bufs=3, space="PSUM"))
        ps_tr = ps_st
        ps_proj = ctx.enter_context(tc.tile_pool(name="ps_proj", bufs=2, space="PSUM"))
        ps_ot = ctx.enter_context(tc.tile_pool(name="ps_ot", bufs=2, space="PSUM"))
        ps_bc = ctx.enter_context(tc.tile_pool(name="ps_bc", bufs=1, space="PSUM"))


        # ---------------- constants ----------------
        # emission order matters for the gpsimd FIFO at startup: identity
        # (tiny, needed by batch-0 transposes), then batch-0's x chunks,
        # then the weights -- so the PE can start transposing ASAP.
        ident = p_const.tile([128, 128], bf16)
        make_identity(nc, ident[:])
        ones_bc = p_const.tile([1, 64], bf16)
        nc.vector.memset(ones_bc[:], 1.0)

        # batch 0's x comes in f32 over the (otherwise idle at startup) sync
        # HWDGE queue and is chunk-cast on the DVE, so the first transposes
        # start ~4us earlier than the gpsimd SWDGE path allows
        x0_f32 = p_x.tile([128, 4, C], f32, tag="x0f", name="x0_f32")
        nc.sync.dma_start(out=x0_f32[:],
                          in_=x_ext[0].rearrange("(nt p) c -> p nt c", p=128))
        x0_sb = p_x.tile([128, 4, C], bf16, tag="x", name="x_sb")
        for ct in range(4):
            nc.vector.tensor_copy(x0_sb[:, :, ct * 128:(ct + 1) * 128],
                                  x0_f32[:, :, ct * 128:(ct + 1) * 128])

        # weights: gpsimd SWDGE cast-DMAs straight to bf16 (no staging, no
        # DVE work); chunked so the DMA queues work in parallel
        wq_sb = p_const.tile([128, 4, E3], bf16)
        wq_r = wq_ext.ap().rearrange("(ct p) e -> p ct e", p=128)
        for ct in range(4):
            nc.gpsimd.dma_start(out=wq_sb[:, ct, :], in_=wq_r[:, ct, :])
        wo_sb = p_const.tile([128, 4, 512], bf16)
        nc.gpsimd.dma_start(
            out=wo_sb[:],
            in_=wo_ext.ap().rearrange("(t p) c -> p t c", p=128))

        # ---------------- per-batch stage emitters ----------------
        def stage_x(b):
            """x [512,512] f32 -> SBUF bf16 (SWDGE cast DMA on idle gpsimd
            queues; keeps the PE all-bf16 so FWL weight loads stay on)."""
            if b == 0:
                return x0_sb
            x_sb = p_x.tile([128, 4, C], bf16, tag="x", name="x_sb")
            nc.gpsimd.dma_start(
                out=x_sb[:],
                in_=x_ext[b].rearrange("(nt p) c -> p nt c", p=128))
            return x_sb

        def stage_prep(b, x_sb):
            """Return (qkT, v_sb, [emission thunks]) for transposes +
            projections of batch b. Thunks are emitted interleaved with the
            previous batch's attention so the PE FIFO stays dense."""
            xT = p_xT.tile([128, 4, N], bf16, tag="xT", name="xT")
            qkT = p_qk.tile([128, 8, N], bf16, tag="qkT", name="qkT")
            v_sb = p_v.tile([128, 4, 8, 65], bf16, tag="v", name="v_sb")
            thunks = []

            def tr(ct):
                if TR_MODE == "dma":
                    # xbar DMA transpose, SBUF->SBUF bf16; no PE, no DVE
                    for nt in range(4):
                        nc.sync.dma_start(
                            out=xT[:, ct, nt * 128:(nt + 1) * 128],
                            in_=x_sb[:, nt, ct * 128:(ct + 1) * 128],
                            transpose=True)
                else:
                    # proj pool (not st): a transpose filler must never
                    # block the PE FIFO on the S^T/exp psum pipeline
                    tr_ps = ps_proj.tile([128, 512], bf16, tag="proj",
                                         name="tr_ps")
                    for nt in range(4):
                        nc.tensor.transpose(
                            tr_ps[:, nt * 128:(nt + 1) * 128],
                            x_sb[:, nt, ct * 128:(ct + 1) * 128], ident[:])
                    nc.vector.tensor_copy(xT[:, ct, :], tr_ps[:])

            def proj_qk(s):
                pr_ps = ps_proj.tile([128, N], f32, tag="proj", name="pr_ps")
                for ct in range(4):
                    nc.tensor.matmul(
                        pr_ps[:],
                        wq_sb[:, ct, s * 128:(s + 1) * 128],
                        xT[:, ct, :],
                        start=(ct == 0), stop=(ct == 3))
                nc.vector.tensor_copy(qkT[:, s, :], pr_ps[:])

            def v_ones():
                nc.vector.memset(v_sb[:, :, :, 64:65], 1.0)

            def proj_v(nt):
                pv_ps = ps_proj.tile([128, N], f32, tag="proj", name="pv_ps")
                for ct in range(4):
                    nc.tensor.matmul(
                        pv_ps[:],
                        xT[:, ct, nt * 128:(nt + 1) * 128],
                        wq_sb[:, ct, 1024:1536],
                        start=(ct == 0), stop=(ct == 3))
                nc.vector.tensor_copy(
                    v_sb[:, nt, :, 0:64],
                    pv_ps[:].rearrange("p (h d) -> p h d", d=64))

            for ct in range(4):
                thunks.append(lambda ct=ct: tr(ct))
            thunks.append(v_ones)
            for s in range(8):
                thunks.append(lambda s=s: proj_qk(s))
            for nt in range(4):
                thunks.append(lambda nt=nt: proj_v(nt))
            return qkT, v_sb, thunks

        def stage_out_units(b, oT):
            """Out-projection as 4 independent filler units + the DMA."""
            out_sb = p_out.tile([128, 4, C], f32, tag="out", name="out_sb")

            def unit(nt):
                f_ps = ps_proj.tile([128, C], f32, tag="proj", name="f_ps")
                for t in range(4):
                    nc.tensor.matmul(
                        f_ps[:],
                        oT[:, t, nt * 128:(nt + 1) * 128],
                        wo_sb[:, t, :],
                        start=(t == 0), stop=(t == 3))
                if ACT_COPIES:
                    nc.scalar.copy(out_sb[:, nt, :], f_ps[:])
                else:
                    nc.vector.tensor_copy(out_sb[:, nt, :], f_ps[:])
                if nt == 3:
                    nc.sync.dma_start(
                        out=out_ext[b].rearrange("(nt p) c -> p nt c", p=128),
                        in_=out_sb[:])

            return [lambda nt=nt: unit(nt) for nt in range(4)]

        def stage_attn(qkT, v_sb, oT, fillers):
            """One batch of attention: per pair-step, the previous pair's PV
            matmuls lead (deps long satisfied), then the current pair's S^T
            matmuls are woven with filler units so the PE FIFO never
            head-of-line blocks on the ACT exp drain."""
            fi = [0]

            def fill(n=1):
                for _ in range(n):
                    if fi[0] < len(fillers):
                        fillers[fi[0]]()
                        fi[0] += 1

            pts_prev = None
            for p in range(5):
                pts_cur = None
                if p < 4:
                    pts_cur = [p_pt.tile([128, 4, N], bf16, tag=f"pt{sub}",
                                         name=f"pt{sub}")
                               for sub in range(2)]
                if p >= 1:
                    pair = p - 1
                    pts = pts_prev
                    bc_ps = ps_bc.tile([128, N], f32, tag="bc", name="bc_ps")
                    ots = []
                    lrows = []
                    for sub in range(2):
                        h = 2 * pair + sub
                        ot_ps = ps_ot.tile([128, N], f32, tag="ot",
                                           name="ot_ps")
                        ots.append(ot_ps)
                        for jt in range(4):
                            nc.tensor.matmul(
                                ot_ps[0:65, :],
                                v_sb[:, jt, h, :],
                                pts[sub][:, jt, :],
                                start=(jt == 0), stop=(jt == 3))
                        lrow = p_small.tile([1, N], bf16, tag=f"lrow{sub}",
                                            name="lrow")
                        nc.vector.tensor_copy(lrow[:], ot_ps[64:65, :])
                        lrows.append(lrow)
                    # give the DVE lrow copies a moment before the PE
                    # broadcast matmuls consume them
                    fill(1)
                    for sub in range(2):
                        nc.tensor.matmul(
                            bc_ps[sub * 64:(sub + 1) * 64, :],
                            ones_bc[:], lrows[sub][:],
                            start=True, stop=True)
                    bc_sb = p_small.tile([128, N], f32, tag="bc_sb",
                                         name="bc_sb")
                    nc.vector.reciprocal_approx_fast(bc_sb[:], bc_ps[:])
                    for sub in range(2):
                        nc.vector.tensor_mul(
                            oT[sub * 64:(sub + 1) * 64, pair, :],
                            ots[sub][0:64, :],
                            bc_sb[sub * 64:(sub + 1) * 64, :])
                if p < 4:
                    for jt in range(4):
                        for sub in range(2):
                            lo, hi = sub * 64, (sub + 1) * 64
                            st_ps = ps_st.tile([128, N], f32, tag="st",
                                               name="st_ps")
                            nc.tensor.matmul(
                                st_ps[:],
                                qkT[lo:hi, 4 + p, jt * 128:(jt + 1) * 128],
                                qkT[lo:hi, p, :],
                                start=True, stop=True)
                            nc.scalar.activation(
                                pts_cur[sub][:, jt, :], st_ps[:], EXP,
                                scale=float(DHEAD) ** -0.5)
                            fill(1)
                pts_prev = pts_cur
            fill(len(fillers))

        # ---------------- cross-batch pipeline ----------------
        # While batch b's attention (ACT-paced) runs, batch b-1's output
        # projection (always-ready filler) and batch b+1's transposes and
        # projections keep the PE FIFO dense between gated S^T matmuls.
        x_sb = stage_x(0)
        qkT, v_sb, prep_thunks = stage_prep(0, x_sb)
        for t in prep_thunks:
            t()
        pending_out = None  # (b, oT) whose out-projection is deferred
        for b in range(BPC):
            fillers = []
            if pending_out is not None:
                fillers += stage_out_units(*pending_out)
                pending_out = None
            if b + 1 < BPC:
                x_next = stage_x(b + 1)
                qkT_n, v_n, next_thunks = stage_prep(b + 1, x_next)
                fillers += next_thunks
            oT = p_oT.tile([128, 4, N], bf16, tag="oT", name="oT")
            stage_attn(qkT, v_sb, oT, fillers)
            pending_out = (b, oT)
            if b + 1 < BPC:
                qkT, v_sb = qkT_n, v_n
        for u in stage_out_units(*pending_out):
            u()

    nc.compile()
    return nc


def _get_nc():
    if "nc" not in _cache:
        _cache["nc"] = _build()
    return _cache["nc"]


def kernel(x, pos_bias=None, w_qkv=None, w_out=None, **_ignored):
    from concourse.bass_utils import run_bass_kernel_spmd

    nc = _get_nc()
    xf = np.ascontiguousarray(np.asarray(x, dtype=np.float32).reshape(B * M, N, C))
    wq = np.ascontiguousarray(np.asarray(w_qkv, dtype=np.float32))
    wo = np.ascontiguousarray(np.asarray(w_out, dtype=np.float32))
    in_maps = [
        {"x": xf[i * BPC:(i + 1) * BPC], "w_qkv": wq, "w_out": wo}
        for i in range(NCORES)
    ]
    res = run_bass_kernel_spmd(
        nc, in_maps, core_ids=list(range(NCORES)),
        trace=bool(_cache.get("trace", False)))
    _cache["last_result"] = res
    out = np.concatenate([res.results[i]["out"] for i in range(NCORES)], axis=0)
    return out.reshape(B, M, N, C).astype(np.float32)



# revision 26
# speedup vs baseline: 1.2190x; 1.2190x over previous
"""Trainium2 Bass kernel for nn_Attention (dense_transformer).

Math (per fused-batch element, 32 total = b*m):
    qkv = x @ w_qkv ; split q,k,v into 8 heads of 64
    sim = (q/8) @ k^T  (+ pos_bias term that is constant along the softmax
                        axis -> provably no effect on softmax output, dropped)
    attn = softmax(sim); out = (attn @ v) heads-concat @ w_out

Sharding: pure data-parallel over the fused (b*m)=32 axis -> 4 elements
per core on 8 cores, no collectives. Weights replicated.

Kernel strategy (per core, all-transposed dataflow, bf16 matmuls):
    xT   = PE-transpose(x)                        [c, n]
    qT,kT (pair-stacked) = W_qk^T @ xT            [e_slice, n]  (psum f32)
    V    = xT-slices @ W_v                        [n, e_v] natural layout,
           stored interleaved [n, h, 65] with a ones column per head
    S^T  = kT_h^T-slice @ qT_h                    [j, i] per head
    P^T  = exp(0.125 * S^T)  (no max subtraction: |logits| <= ~8)
    outT_h (rows 0..63) + L_h (row 64) = V1_h^T @ P^T   (ones-column trick)
    OT   = outT_h * broadcast(1/L_h)  (K=1 matmul broadcast + DVE mul)
    out  = OT-slices^T @ w_out        [n, c] -> DMA out
"""

import os
import sys

for _p in ("/root/.axon_site/_ro/trn_rl_repo", "/opt/trn_rl_repo"):
    if os.path.isdir(_p) and _p not in sys.path:
        sys.path.append(_p)

import numpy as np

# ---- problem constants (hardcoded per spec) ----
B, M, N, C = 4, 8, 512, 512
HEADS, DHEAD = 8, 64
E3 = 3 * 512
NCORES = 8
BPC = (B * M) // NCORES  # batch elements per core = 4
BCAST_MODE = "gpsimd"  # "gpsimd" | "pe"
RECIP_MODE = "approx_sbuf"  # "exact" | "approx_sbuf" | "approx_psum"
TR_MODE = "pe"  # "dma" (xbar transpose) | "pe" (tensor-engine transpose)
ACT_COPIES = False  # offload some psum->sbuf copies to the Scalar engine

_cache = {}


def _build():
    import concourse.bass as bass
    import concourse.mybir as mybir
    import concourse.tile as tile
    from concourse import bacc
    from concourse.masks import make_identity

    f32 = mybir.dt.float32
    bf16 = mybir.dt.bfloat16
    f32r = mybir.dt.float32r
    EXP = mybir.ActivationFunctionType.Exp

    nc = bacc.Bacc("TRN2", target_bir_lowering=False, debug=False,
                   num_devices=NCORES)

    x_ext = nc.declare_dram_parameter("x", [BPC, N, C], f32, isOutput=False)
    wq_ext = nc.declare_dram_parameter("w_qkv", [C, E3], f32, isOutput=False)
    wo_ext = nc.declare_dram_parameter("w_out", [512, 512], f32, isOutput=False)
    out_ext = nc.declare_dram_parameter("out", [BPC, N, C], f32, isOutput=True)

    from contextlib import ExitStack

    with tile.TileContext(nc) as tc, ExitStack() as ctx:
        # ---------------- pools ----------------
        p_const = ctx.enter_context(tc.tile_pool(name="const", bufs=1))
        p_stage = ctx.enter_context(tc.tile_pool(name="stage", bufs=1))
        p_x = ctx.enter_context(tc.tile_pool(name="x", bufs=2))
        p_xT = ctx.enter_context(tc.tile_pool(name="xT", bufs=2))
        p_qk = ctx.enter_context(tc.tile_pool(name="qk", bufs=2))
        p_v = ctx.enter_context(tc.tile_pool(name="v", bufs=2))
        p_pt = ctx.enter_context(tc.tile_pool(name="pt", bufs=4))
        p_oT = ctx.enter_context(tc.tile_pool(name="oT", bufs=2))
        p_out = ctx.enter_context(tc.tile_pool(name="out", bufs=2))
        p_small = ctx.enter_context(tc.tile_pool(name="small", bufs=4))

        # tr and st share one pool (same tag) so STs can run well ahead of
        # the ACT exps; PSUM slots are allocated dynamically from the shared
        # 8-bank free pool, so nominal bufs sums may exceed 8
        ps_st = ctx.enter_context(tc.tile_pool(name="ps_st", bufs=4, space="PSUM"))
        ps_tr = ps_st
        ps_proj = ctx.enter_context(tc.tile_pool(name="ps_proj", bufs=2, space="PSUM"))
        ps_ot = ctx.enter_context(tc.tile_pool(name="ps_ot", bufs=2, space="PSUM"))


        # ---------------- constants ----------------
        # emission order matters for the gpsimd FIFO at startup: identity
        # (tiny, needed by batch-0 transposes), then batch-0's x chunks,
        # then the weights -- so the PE can start transposing ASAP.
        ident = p_const.tile([128, 128], bf16)
        make_identity(nc, ident[:])
        ones_bc = p_const.tile([1, 64], bf16)
        nc.vector.memset(ones_bc[:], 1.0)

        # batch 0's x comes in f32 over the (otherwise idle at startup) sync
        # HWDGE queue and is chunk-cast on the DVE, so the first transposes
        # start ~4us earlier than the gpsimd SWDGE path allows
        x0_f32 = p_x.tile([128, 4, C], f32, tag="x0f", name="x0_f32")
        nc.sync.dma_start(out=x0_f32[:],
                          in_=x_ext[0].rearrange("(nt p) c -> p nt c", p=128))
        x0_sb = p_x.tile([128, 4, C], bf16, tag="x", name="x_sb")
        for ct in range(4):
            nc.vector.tensor_copy(x0_sb[:, :, ct * 128:(ct + 1) * 128],
                                  x0_f32[:, :, ct * 128:(ct + 1) * 128])

        # weights: gpsimd SWDGE cast-DMAs straight to bf16 (no staging, no
        # DVE work); chunked so the DMA queues work in parallel
        wq_sb = p_const.tile([128, 4, E3], bf16)
        wq_r = wq_ext.ap().rearrange("(ct p) e -> p ct e", p=128)
        for ct in range(4):
            nc.gpsimd.dma_start(out=wq_sb[:, ct, :], in_=wq_r[:, ct, :])
        wo_sb = p_const.tile([128, 4, 512], bf16)
        nc.gpsimd.dma_start(
            out=wo_sb[:],
            in_=wo_ext.ap().rearrange("(t p) c -> p t c", p=128))

        # ---------------- per-batch stage emitters ----------------
        def stage_x(b):
            """x [512,512] f32 -> SBUF bf16 (SWDGE cast DMA on idle gpsimd
            queues; keeps the PE all-bf16 so FWL weight loads stay on)."""
            if b == 0:
                return x0_sb
            x_sb = p_x.tile([128, 4, C], bf16, tag="x", name="x_sb")
            nc.gpsimd.dma_start(
                out=x_sb[:],
                in_=x_ext[b].rearrange("(nt p) c -> p nt c", p=128))
            return x_sb

        def stage_prep(b, x_sb):
            """Return (qkT, v_sb, [emission thunks]) for transposes +
            projections of batch b. Thunks are emitted interleaved with the
            previous batch's attention so the PE FIFO stays dense."""
            xT = p_xT.tile([128, 4, N], bf16, tag="xT", name="xT")
            qkT = p_qk.tile([128, 8, N], bf16, tag="qkT", name="qkT")
            v_sb = p_v.tile([128, 4, 8, 65], bf16, tag="v", name="v_sb")
            thunks = []

            def tr(ct):
                if TR_MODE == "dma":
                    # xbar DMA transpose, SBUF->SBUF bf16; no PE, no DVE
                    for nt in range(4):
                        nc.sync.dma_start(
                            out=xT[:, ct, nt * 128:(nt + 1) * 128],
                            in_=x_sb[:, nt, ct * 128:(ct + 1) * 128],
                            transpose=True)
                else:
                    # proj pool (not st): a transpose filler must never
                    # block the PE FIFO on the S^T/exp psum pipeline
                    tr_ps = ps_proj.tile([128, 512], bf16, tag="proj",
                                         name="tr_ps")
                    for nt in range(4):
                        nc.tensor.transpose(
                            tr_ps[:, nt * 128:(nt + 1) * 128],
                            x_sb[:, nt, ct * 128:(ct + 1) * 128], ident[:])
                    nc.vector.tensor_copy(xT[:, ct, :], tr_ps[:])

            def proj_qk(s):
                pr_ps = ps_proj.tile([128, N], f32, tag="proj", name="pr_ps")
                for ct in range(4):
                    nc.tensor.matmul(
                        pr_ps[:],
                        wq_sb[:, ct, s * 128:(s + 1) * 128],
                        xT[:, ct, :],
                        start=(ct == 0), stop=(ct == 3))
                nc.vector.tensor_copy(qkT[:, s, :], pr_ps[:])

            def v_ones():
                nc.vector.memset(v_sb[:, :, :, 64:65], 1.0)

            def proj_v(nt):
                pv_ps = ps_proj.tile([128, N], f32, tag="proj", name="pv_ps")
                for ct in range(4):
                    nc.tensor.matmul(
                        pv_ps[:],
                        xT[:, ct, nt * 128:(nt + 1) * 128],
                        wq_sb[:, ct, 1024:1536],
                        start=(ct == 0), stop=(ct == 3))
                nc.vector.tensor_copy(
                    v_sb[:, nt, :, 0:64],
                    pv_ps[:].rearrange("p (h d) -> p h d", d=64))

            for ct in range(4):
                thunks.append(lambda ct=ct: tr(ct))
            thunks.append(v_ones)
            for s in range(8):
                thunks.append(lambda s=s: proj_qk(s))
            for nt in range(4):
                thunks.append(lambda nt=nt: proj_v(nt))
            return qkT, v_sb, thunks

        def stage_out_units(b, oT):
            """Out-projection as 4 independent filler units + the DMA."""
            out_sb = p_out.tile([128, 4, C], f32, tag="out", name="out_sb")

            def unit(nt):
                f_ps = ps_proj.tile([128, C], f32, tag="proj", name="f_ps")
                for t in range(4):
                    nc.tensor.matmul(
                        f_ps[:],
                        oT[:, t, nt * 128:(nt + 1) * 128],
                        wo_sb[:, t, :],
                        start=(t == 0), stop=(t == 3))
                if ACT_COPIES:
                    nc.scalar.copy(out_sb[:, nt, :], f_ps[:])
                else:
                    nc.vector.tensor_copy(out_sb[:, nt, :], f_ps[:])
                if nt == 3:
                    nc.sync.dma_start(
                        out=out_ext[b].rearrange("(nt p) c -> p nt c", p=128),
                        in_=out_sb[:])

            return [lambda nt=nt: unit(nt) for nt in range(4)]

        # ---------------- flat global pair pipeline ----------------
        # Global pair index g: S^T/exp of pair g runs interleaved with the
        # PV/normalize of pair g-1 at individual-matmul granularity, so the
        # ACT exp stream never bubbles (st#1 of pair g issues as soon as its
        # PSUM bank frees, ~3 exps before pair g-1's drain completes) and
        # the PE FIFO always has ready work queued behind gated matmuls.
        from collections import deque

        fillers = deque()

        def fill(n=1):
            for _ in range(n):
                if fillers:
                    fillers.popleft()()

        NPAIR = 4 * BPC
        qkT_by_b = {}
        v_by_b = {}
        oT_by_b = {}
        pts_prev = None

        x_sb = stage_x(0)
        qkT_by_b[0], v_by_b[0], prep0 = stage_prep(0, x_sb)
        for t in prep0:
            t()

        for g in range(NPAIR + 1):
            b_st, p_st = divmod(g, 4)
            do_st = g < NPAIR
            do_pv = g >= 1
            if do_pv:
                bpv, ppv = divmod(g - 1, 4)
                if ppv == 0:
                    oT_by_b[bpv] = p_oT.tile([128, 4, N], bf16, tag="oT",
                                             name="oT")
                oT = oT_by_b[bpv]
                v_sb = v_by_b[bpv]
                pts = pts_prev

            # batch-boundary events feed the filler queue
            if do_st and p_st == 0 and b_st + 1 < BPC:
                x_next = stage_x(b_st + 1)
                qkT_by_b[b_st + 1], v_by_b[b_st + 1], prep_n = \
                    stage_prep(b_st + 1, x_next)
                fillers.extend(prep_n)
            # out-proj of batch b-1 becomes eligible only after step 4b's
            # normalize has written its final oT pair (emission order IS the
            # dependency order for the PE FIFO)
            if do_st and p_st == 1 and b_st >= 1:
                fillers.extend(stage_out_units(b_st - 1,
                                               oT_by_b.pop(b_st - 1)))

            pts_cur = None
            if do_st:
                qkT = qkT_by_b[b_st]
                pts_cur = [p_pt.tile([128, 4, N], bf16, tag=f"pt{sub}",
                                     name=f"pt{sub}")
                           for sub in range(2)]

            ots = [None, None]
            lrows = [None, None]

            def pv(sub, jt):
                h = 2 * ppv + sub
                if jt == 0:
                    ots[sub] = ps_ot.tile([128, N], f32, tag="ot",
                                          name="ot_ps")
                nc.tensor.matmul(
                    ots[sub][0:65, :],
                    v_sb[:, jt, h, :],
                    pts[sub][:, jt, :],
                    start=(jt == 0), stop=(jt == 3))

            def lrow_copy(sub):
                lrows[sub] = p_small.tile([1, N], bf16, tag=f"lrow{sub}",
                                          name="lrow")
                nc.vector.tensor_copy(lrows[sub][:], ots[sub][64:65, :])

            def st(k):
                jt, sub = divmod(k, 2)
                lo, hi = sub * 64, (sub + 1) * 64
                st_ps = ps_st.tile([128, N], f32, tag="st", name="st_ps")
                nc.tensor.matmul(
                    st_ps[:],
                    qkT[lo:hi, 4 + p_st, jt * 128:(jt + 1) * 128],
                    qkT[lo:hi, p_st, :],
                    start=True, stop=True)
                nc.scalar.activation(
                    pts_cur[sub][:, jt, :], st_ps[:], EXP,
                    scale=float(DHEAD) ** -0.5)

            def normalize():
                # broadcast each head's L down 64 partitions of a fresh bank
                # (K=1 matmul), then per-sub reciprocal + multiply
                bcs = []
                for sub in range(2):
                    bc_ps = ps_proj.tile([128, N], f32, tag="proj",
                                         name="bc_ps")
                    bcs.append(bc_ps)
                    nc.tensor.matmul(
                        bc_ps[0:64, :],
                        ones_bc[:], lrows[sub][:],
                        start=True, stop=True)
                for sub in range(2):
                    bc_sb = p_small.tile([64, N], f32, tag=f"bc{sub}",
                                         name="bc_sb")
                    nc.vector.reciprocal_approx_fast(
                        bc_sb[:], bcs[sub][0:64, :])
                    nc.vector.tensor_mul(
                        oT[sub * 64:(sub + 1) * 64, ppv, :],
                        ots[sub][0:64, :], bc_sb[:])

            # ---- the interleave ----
            if do_pv:
                pv(0, 0); pv(0, 1); pv(0, 2); pv(0, 3)
                lrow_copy(0)
                pv(1, 0); pv(1, 1); pv(1, 2); pv(1, 3)
                lrow_copy(1)
                fill(1)
                normalize()
            if do_st:
                for k in range(8):
                    st(k)
                    fill(1)
            else:
                fill(2)
            pts_prev = pts_cur

        while fillers:
            fillers.popleft()()
        for u in stage_out_units(BPC - 1, oT_by_b.pop(BPC - 1)):
            u()

    nc.compile()
    return nc


def _get_nc():
    if "nc" not in _cache:
        _cache["nc"] = _build()
    return _cache["nc"]


def kernel(x, pos_bias=None, w_qkv=None, w_out=None, **_ignored):
    from concourse.bass_utils import run_bass_kernel_spmd

    nc = _get_nc()
    xf = np.ascontiguousarray(np.asarray(x, dtype=np.float32).reshape(B * M, N, C))
    wq = np.ascontiguousarray(np.asarray(w_qkv, dtype=np.float32))
    wo = np.ascontiguousarray(np.asarray(w_out, dtype=np.float32))
    in_maps = [
        {"x": xf[i * BPC:(i + 1) * BPC], "w_qkv": wq, "w_out": wo}
        for i in range(NCORES)
    ]
    res = run_bass_kernel_spmd(
        nc, in_maps, core_ids=list(range(NCORES)),
        trace=bool(_cache.get("trace", False)))
    _cache["last_result"] = res
    out = np.concatenate([res.results[i]["out"] for i in range(NCORES)], axis=0)
    return out.reshape(B, M, N, C).astype(np.float32)



# revision 34
# speedup vs baseline: 1.2737x; 1.0449x over previous
"""Trainium2 Bass kernel for nn_Attention (dense_transformer).

Math (per fused-batch element, 32 total = b*m):
    qkv = x @ w_qkv ; split q,k,v into 8 heads of 64
    sim = (q/8) @ k^T  (+ pos_bias term that is constant along the softmax
                        axis -> provably no effect on softmax output, dropped)
    attn = softmax(sim); out = (attn @ v) heads-concat @ w_out

Sharding: pure data-parallel over the fused (b*m)=32 axis -> 4 elements
per core on 8 cores, no collectives. Weights replicated.

Kernel strategy (per core, all-transposed dataflow, bf16 matmuls):
    xT   = PE-transpose(x)                        [c, n]
    qT,kT (pair-stacked) = W_qk^T @ xT            [e_slice, n]  (psum f32)
    V    = xT-slices @ W_v                        [n, e_v] natural layout,
           stored interleaved [n, h, 65] with a ones column per head
    S^T  = kT_h^T-slice @ qT_h                    [j, i] per head
    P^T  = exp(0.125 * S^T)  (no max subtraction: |logits| <= ~8)
    outT_h (rows 0..63) + L_h (row 64) = V1_h^T @ P^T   (ones-column trick)
    OT   = outT_h * broadcast(1/L_h)  (K=1 matmul broadcast + DVE mul)
    out  = OT-slices^T @ w_out        [n, c] -> DMA out
"""

import os
import sys

for _p in ("/root/.axon_site/_ro/trn_rl_repo", "/opt/trn_rl_repo"):
    if os.path.isdir(_p) and _p not in sys.path:
        sys.path.append(_p)

import numpy as np

# ---- problem constants (hardcoded per spec) ----
B, M, N, C = 4, 8, 512, 512
HEADS, DHEAD = 8, 64
E3 = 3 * 512
NCORES = 8
BPC = (B * M) // NCORES  # batch elements per core = 4
BCAST_MODE = "gpsimd"  # "gpsimd" | "pe"
RECIP_MODE = "approx_sbuf"  # "exact" | "approx_sbuf" | "approx_psum"
TR_MODE = "pe"  # "dma" (xbar transpose) | "pe" (tensor-engine transpose)
ACT_COPIES = False  # offload some psum->sbuf copies to the Scalar engine

_cache = {}


def _build():
    import concourse.bass as bass
    import concourse.mybir as mybir
    import concourse.tile as tile
    from concourse import bacc
    from concourse.masks import make_identity

    f32 = mybir.dt.float32
    bf16 = mybir.dt.bfloat16
    f32r = mybir.dt.float32r
    EXP = mybir.ActivationFunctionType.Exp

    nc = bacc.Bacc("TRN2", target_bir_lowering=False, debug=False,
                   num_devices=NCORES)

    x_ext = nc.declare_dram_parameter("x", [BPC, N, C], f32, isOutput=False)
    wq_ext = nc.declare_dram_parameter("w_qkv", [C, E3], f32, isOutput=False)
    wo_ext = nc.declare_dram_parameter("w_out", [512, 512], f32, isOutput=False)
    out_ext = nc.declare_dram_parameter("out", [BPC, N, C], f32, isOutput=True)

    from contextlib import ExitStack

    with tile.TileContext(nc) as tc, ExitStack() as ctx:
        # ---------------- pools ----------------
        p_const = ctx.enter_context(tc.tile_pool(name="const", bufs=1))
        p_stage = ctx.enter_context(tc.tile_pool(name="stage", bufs=1))
        p_x = ctx.enter_context(tc.tile_pool(name="x", bufs=2))
        p_xT = ctx.enter_context(tc.tile_pool(name="xT", bufs=2))
        p_qk = ctx.enter_context(tc.tile_pool(name="qk", bufs=2))
        p_v = ctx.enter_context(tc.tile_pool(name="v", bufs=2))
        p_pt = ctx.enter_context(tc.tile_pool(name="pt", bufs=4))
        p_oT = ctx.enter_context(tc.tile_pool(name="oT", bufs=1))
        p_out = ctx.enter_context(tc.tile_pool(name="out", bufs=2))
        p_small = ctx.enter_context(tc.tile_pool(name="small", bufs=4))

        # tr and st share one pool (same tag) so STs can run well ahead of
        # the ACT exps; PSUM slots are allocated dynamically from the shared
        # 8-bank free pool, so nominal bufs sums may exceed 8
        ps_st = ctx.enter_context(tc.tile_pool(name="ps_st", bufs=4, space="PSUM"))
        ps_tr = ps_st
        ps_proj = ctx.enter_context(tc.tile_pool(name="ps_proj", bufs=2, space="PSUM"))
        ps_ot = ctx.enter_context(tc.tile_pool(name="ps_ot", bufs=1, space="PSUM"))


        # ---------------- constants ----------------
        # emission order matters for the gpsimd FIFO at startup: identity
        # (tiny, needed by batch-0 transposes), then batch-0's x chunks,
        # then the weights -- so the PE can start transposing ASAP.
        ident = p_const.tile([128, 128], bf16)
        make_identity(nc, ident[:])
        ones_bc = p_const.tile([1, 64], bf16)
        nc.vector.memset(ones_bc[:], 1.0)

        # batch 0's x comes in f32 over the (otherwise idle at startup) sync
        # HWDGE queue and is chunk-cast on the DVE, so the first transposes
        # start ~4us earlier than the gpsimd SWDGE path allows
        x0_f32 = p_x.tile([128, 4, C], f32, tag="x0f", name="x0_f32")
        nc.sync.dma_start(out=x0_f32[:],
                          in_=x_ext[0].rearrange("(nt p) c -> p nt c", p=128))
        x0_sb = p_x.tile([128, 4, C], bf16, tag="x", name="x_sb")
        for ct in range(4):
            nc.vector.tensor_copy(x0_sb[:, :, ct * 128:(ct + 1) * 128],
                                  x0_f32[:, :, ct * 128:(ct + 1) * 128])

        # weights: gpsimd SWDGE cast-DMAs straight to bf16 (no staging, no
        # DVE work); chunked so the DMA queues work in parallel
        wq_sb = p_const.tile([128, 4, E3], bf16)
        wq_r = wq_ext.ap().rearrange("(ct p) e -> p ct e", p=128)
        for ct in range(4):
            nc.gpsimd.dma_start(out=wq_sb[:, ct, :], in_=wq_r[:, ct, :])
        wo_sb = p_const.tile([128, 4, 512], bf16)
        nc.gpsimd.dma_start(
            out=wo_sb[:],
            in_=wo_ext.ap().rearrange("(t p) c -> p t c", p=128))

        # ---------------- per-batch stage emitters ----------------
        def stage_x(b):
            """x [512,512] f32 -> SBUF bf16 (SWDGE cast DMA on idle gpsimd
            queues; keeps the PE all-bf16 so FWL weight loads stay on)."""
            if b == 0:
                return x0_sb
            x_sb = p_x.tile([128, 4, C], bf16, tag="x", name="x_sb")
            nc.gpsimd.dma_start(
                out=x_sb[:],
                in_=x_ext[b].rearrange("(nt p) c -> p nt c", p=128))
            return x_sb

        def stage_prep(b, x_sb):
            """Return (qkT, v_sb, [emission thunks]) for transposes +
            projections of batch b. Thunks are emitted interleaved with the
            previous batch's attention so the PE FIFO stays dense."""
            xT = p_xT.tile([128, 4, N], bf16, tag="xT", name="xT")
            qkT = p_qk.tile([128, 8, N], bf16, tag="qkT", name="qkT")
            v_sb = p_v.tile([128, 4, 8, 65], bf16, tag="v", name="v_sb")
            thunks = []

            def tr(ct):
                if TR_MODE == "dma":
                    # xbar DMA transpose, SBUF->SBUF bf16; no PE, no DVE
                    for nt in range(4):
                        nc.sync.dma_start(
                            out=xT[:, ct, nt * 128:(nt + 1) * 128],
                            in_=x_sb[:, nt, ct * 128:(ct + 1) * 128],
                            transpose=True)
                else:
                    # proj pool (not st): a transpose filler must never
                    # block the PE FIFO on the S^T/exp psum pipeline
                    tr_ps = ps_proj.tile([128, 512], bf16, tag="proj",
                                         name="tr_ps")
                    for nt in range(4):
                        nc.tensor.transpose(
                            tr_ps[:, nt * 128:(nt + 1) * 128],
                            x_sb[:, nt, ct * 128:(ct + 1) * 128], ident[:])
                    nc.vector.tensor_copy(xT[:, ct, :], tr_ps[:])

            def proj_qk(s):
                pr_ps = ps_proj.tile([128, N], f32, tag="proj", name="pr_ps")
                for ct in range(4):
                    nc.tensor.matmul(
                        pr_ps[:],
                        wq_sb[:, ct, s * 128:(s + 1) * 128],
                        xT[:, ct, :],
                        start=(ct == 0), stop=(ct == 3))
                nc.vector.tensor_copy(qkT[:, s, :], pr_ps[:])

            def v_ones():
                nc.vector.memset(v_sb[:, :, :, 64:65], 1.0)

            def proj_v(nt):
                pv_ps = ps_proj.tile([128, N], f32, tag="proj", name="pv_ps")
                for ct in range(4):
                    nc.tensor.matmul(
                        pv_ps[:],
                        xT[:, ct, nt * 128:(nt + 1) * 128],
                        wq_sb[:, ct, 1024:1536],
                        start=(ct == 0), stop=(ct == 3))
                nc.vector.tensor_copy(
                    v_sb[:, nt, :, 0:64],
                    pv_ps[:].rearrange("p (h d) -> p h d", d=64))

            for ct in range(4):
                thunks.append(lambda ct=ct: tr(ct))
            thunks.append(v_ones)
            for s in range(8):
                thunks.append(lambda s=s: proj_qk(s))
            for nt in range(4):
                thunks.append(lambda nt=nt: proj_v(nt))
            return qkT, v_sb, thunks

        def stage_out_units(b, oT):
            """Out-projection as 4 independent filler units + the DMA."""
            out_sb = p_out.tile([128, 4, C], f32, tag="out", name="out_sb")

            def unit(nt):
                f_ps = ps_proj.tile([128, C], f32, tag="proj", name="f_ps")
                for t in range(4):
                    nc.tensor.matmul(
                        f_ps[:],
                        oT[:, t, nt * 128:(nt + 1) * 128],
                        wo_sb[:, t, :],
                        start=(t == 0), stop=(t == 3))
                if ACT_COPIES:
                    nc.scalar.copy(out_sb[:, nt, :], f_ps[:])
                else:
                    nc.vector.tensor_copy(out_sb[:, nt, :], f_ps[:])
                if nt == 3:
                    nc.sync.dma_start(
                        out=out_ext[b].rearrange("(nt p) c -> p nt c", p=128),
                        in_=out_sb[:])

            return [lambda nt=nt: unit(nt) for nt in range(4)]

        # ---------------- flat global pair pipeline ----------------
        # Global pair index g: S^T/exp of pair g runs interleaved with the
        # PV/normalize of pair g-1 at individual-matmul granularity, so the
        # ACT exp stream never bubbles (st#1 of pair g issues as soon as its
        # PSUM bank frees, ~3 exps before pair g-1's drain completes) and
        # the PE FIFO always has ready work queued behind gated matmuls.
        from collections import deque

        fillers = deque()

        def fill(n=1):
            for _ in range(n):
                if fillers:
                    fillers.popleft()()

        NPAIR = 4 * BPC
        qkT_by_b = {}
        v_by_b = {}
        oT_by_b = {}
        pts_prev = None

        # fixed PV psum tiles (one per sub) and fixed ping-pong oT tiles
        # (by batch parity): WAR data-deps instead of pool slot-waits, so
        # the list scheduler can never park an engine on a slot wait
        ot_fixed = [ps_ot.tile([128, N], f32, tag=f"ot{s}", name=f"ot{s}")
                    for s in range(2)]
        oT_fixed = [p_oT.tile([128, 4, N], bf16, tag=f"oT{i}", name=f"oT{i}")
                    for i in range(2)]

        x_sb = stage_x(0)
        qkT_by_b[0], v_by_b[0], prep0 = stage_prep(0, x_sb)
        for t in prep0:
            t()

        for g in range(NPAIR + 1):
            b_st, p_st = divmod(g, 4)
            do_st = g < NPAIR
            do_pv = g >= 1
            if do_pv:
                bpv, ppv = divmod(g - 1, 4)
                if ppv == 0:
                    oT_by_b[bpv] = oT_fixed[bpv % 2]
                oT = oT_by_b[bpv]
                v_sb = v_by_b[bpv]
                pts = pts_prev

            # batch-boundary events feed the filler queue
            if do_st and p_st == 0 and b_st + 1 < BPC:
                x_next_by_b = stage_x(b_st + 1)
            # prep(b+1) and out(b-1) are deferred to p_st==1: both alias
            # (via pool rotation / oT ping-pong) memory whose final readers
            # and writers are only emitted with pair 4b-1's PV in step 4b --
            # pool release points cover only readers emitted so far
            if do_st and p_st == 1:
                if b_st >= 1:
                    fillers.extend(stage_out_units(b_st - 1,
                                                   oT_by_b.pop(b_st - 1)))
                if b_st + 1 < BPC:
                    qkT_by_b[b_st + 1], v_by_b[b_st + 1], prep_n = \
                        stage_prep(b_st + 1, x_next_by_b)
                    fillers.extend(prep_n)

            pts_cur = None
            if do_st:
                qkT = qkT_by_b[b_st]
                pts_cur = [p_pt.tile([128, 4, N], bf16, tag=f"pt{sub}",
                                     name=f"pt{sub}")
                           for sub in range(2)]

            ots = [None, None]
            lrows = [None, None]

            def pv(sub, jt):
                h = 2 * ppv + sub
                if jt == 0:
                    ots[sub] = ot_fixed[sub]
                nc.tensor.matmul(
                    ots[sub][0:65, :],
                    v_sb[:, jt, h, :],
                    pts[sub][:, jt, :],
                    start=(jt == 0), stop=(jt == 3))

            def lrow_copy(sub):
                lrows[sub] = p_small.tile([1, N], bf16, tag=f"lrow{sub}",
                                          name="lrow")
                nc.vector.tensor_copy(lrows[sub][:], ots[sub][64:65, :])

            def st(k):
                jt, sub = divmod(k, 2)
                lo, hi = sub * 64, (sub + 1) * 64
                st_ps = ps_st.tile([128, N], f32, tag="st", name="st_ps")
                nc.tensor.matmul(
                    st_ps[:],
                    qkT[lo:hi, 4 + p_st, jt * 128:(jt + 1) * 128],
                    qkT[lo:hi, p_st, :],
                    start=True, stop=True)
                nc.scalar.activation(
                    pts_cur[sub][:, jt, :], st_ps[:], EXP,
                    scale=float(DHEAD) ** -0.5)

            def normalize():
                # broadcast each head's L down 64 partitions of a fresh bank
                # (K=1 matmul), then per-sub reciprocal + multiply
                bcs = []
                for sub in range(2):
                    bc_ps = ps_proj.tile([128, N], f32, tag="proj",
                                         name="bc_ps")
                    bcs.append(bc_ps)
                    nc.tensor.matmul(
                        bc_ps[0:64, :],
                        ones_bc[:], lrows[sub][:],
                        start=True, stop=True)
                for sub in range(2):
                    bc_sb = p_small.tile([64, N], f32, tag=f"bc{sub}",
                                         name="bc_sb")
                    nc.vector.reciprocal_approx_fast(
                        bc_sb[:], bcs[sub][0:64, :])
                    nc.vector.tensor_mul(
                        oT[sub * 64:(sub + 1) * 64, ppv, :],
                        ots[sub][0:64, :], bc_sb[:])

            # ---- the interleave ----
            # sts lead so the ACT exp stream never bubbles; each pv
            # accumulation group stays contiguous within its own bank
            if do_st:
                st(0)
                fill(1)
                st(1)
                fill(1)
            if do_pv:
                pv(0, 0); pv(0, 1); pv(0, 2); pv(0, 3)
                lrow_copy(0)
            if do_st:
                st(2)
                fill(1)
            if do_pv:
                pv(1, 0); pv(1, 1); pv(1, 2); pv(1, 3)
                lrow_copy(1)
            if do_st:
                st(3)
                fill(1)
            if do_pv:
                normalize()
            else:
                fill(1)
            if do_st:
                for k in range(4, 8):
                    st(k)
                    fill(1)
            else:
                fill(3)
            pts_prev = pts_cur

        while fillers:
            fillers.popleft()()
        for u in stage_out_units(BPC - 1, oT_by_b.pop(BPC - 1)):
            u()

    nc.compile()
    return nc


def _get_nc():
    if "nc" not in _cache:
        _cache["nc"] = _build()
    return _cache["nc"]


def kernel(x, pos_bias=None, w_qkv=None, w_out=None, **_ignored):
    from concourse.bass_utils import run_bass_kernel_spmd

    nc = _get_nc()
    xf = np.ascontiguousarray(np.asarray(x, dtype=np.float32).reshape(B * M, N, C))
    wq = np.ascontiguousarray(np.asarray(w_qkv, dtype=np.float32))
    wo = np.ascontiguousarray(np.asarray(w_out, dtype=np.float32))
    in_maps = [
        {"x": xf[i * BPC:(i + 1) * BPC], "w_qkv": wq, "w_out": wo}
        for i in range(NCORES)
    ]
    res = run_bass_kernel_spmd(
        nc, in_maps, core_ids=list(range(NCORES)),
        trace=bool(_cache.get("trace", False)))
    _cache["last_result"] = res
    out = np.concatenate([res.results[i]["out"] for i in range(NCORES)], axis=0)
    return out.reshape(B, M, N, C).astype(np.float32)

